# revision 3
# baseline (speedup 1.0000x reference)
"""Trainium2 Bass kernel for nn_EquivarianceNetwork (grouped 4-layer MLP).

Math (per sample b, TWO_N=16 groups, D=64):
  xr = x.reshape(B, 16, 64)
  scalars[b, n, m] = <xr[b,n], xr[b,m]>                  # [B, 256]
  per group l: h = tanh(...W0/W1/W2...), coeffs = h @ W3 + b3   # [B, 16]
  out[b, l*64:(l+1)*64] = sum_n coeffs[l,b,n] * xr[b,n]

Distribution: data-parallel over batch across 8 cores (weights replicated).
Per core B_local = 2048.

Engine plan per core:
  - Gram + final contraction: DVE (fp32), batch-major subtiles [128, 1024].
  - MLP GEMMs: PE in float32r (fp32 data, TF32-like matmul precision,
    1 cycle/row at N=512), activations kept feature-major [feat, batch].
  - tanh + bias: ACT, reading PSUM, writing float32r SBUF tiles.
  - Weights streamed from HBM per group l, double-buffered.
"""
import numpy as np

import concourse.bass as bass
import concourse.mybir as mybir
import concourse.tile as tile
from concourse import bacc
from concourse.bass_utils import run_bass_kernel_spmd
from concourse.masks import make_identity

F32 = mybir.dt.float32
F32R = mybir.dt.float32r

N_CORES = 8
B = 16384
TWO_N = 16
D = 64
B_LOC = B // N_CORES          # 2048
N_SUB = B_LOC // 128          # 16 subtiles of 128 samples
N_BT = B_LOC // 512           # 4 batch tiles of 512 (matmul free dim)
H = 1024                      # hidden width
K_IN = 256                    # 16*16 scalars


def _build_program():
    nc = bacc.Bacc()

    x = nc.declare_dram_parameter("x", [B_LOC, TWO_N * D], F32, isOutput=False)
    W0 = nc.declare_dram_parameter("W0", [TWO_N, K_IN, H], F32R, isOutput=False)
    W1 = nc.declare_dram_parameter("W1", [TWO_N, H, H], F32R, isOutput=False)
    W2 = nc.declare_dram_parameter("W2", [TWO_N, H, H], F32R, isOutput=False)
    W3 = nc.declare_dram_parameter("W3", [TWO_N, H, TWO_N], F32R, isOutput=False)
    b0 = nc.declare_dram_parameter("b0", [TWO_N, H], F32, isOutput=False)
    b1 = nc.declare_dram_parameter("b1", [TWO_N, H], F32, isOutput=False)
    b2 = nc.declare_dram_parameter("b2", [TWO_N, H], F32, isOutput=False)
    b3 = nc.declare_dram_parameter("b3", [TWO_N, TWO_N], F32, isOutput=False)
    y = nc.declare_dram_parameter("y", [B_LOC, TWO_N * D], F32, isOutput=True)

    with tile.TileContext(nc) as tc:
        with tc.tile_pool(name="res", bufs=1) as res, \
             tc.tile_pool(name="xg", bufs=3) as xgp, \
             tc.tile_pool(name="work", bufs=2) as wk, \
             tc.tile_pool(name="w0p", bufs=2) as w0p, \
             tc.tile_pool(name="w12p", bufs=5) as w12p, \
             tc.tile_pool(name="w3p", bufs=2) as w3p, \
             tc.tile_pool(name="bp", bufs=2) as bp, \
             tc.tile_pool(name="hp", bufs=2) as hp, \
             tc.tile_pool(name="fin", bufs=4) as finp, \
             tc.tile_pool(name="ps", bufs=4, space="PSUM") as ps:

            ident = res.tile([128, 128], F32)
            make_identity(nc, ident)

            # resident: transposed scalars [256, B_LOC] as 2 partition tiles
            scalT = [res.tile([128, B_LOC], F32R, name=f"scalT{i}")
                     for i in range(2)]
            # resident: coeffs batch-major per subtile [128, 256] (col l*16+n)
            coeff = [res.tile([128, 256], F32, name=f"coeff{s}")
                     for s in range(N_SUB)]

            # ---------------- Phase A: Gram matrices ----------------
            for s in range(N_SUB):
                xg = xgp.tile([128, TWO_N * D], F32, name="xg", tag="xg")
                nc.sync.dma_start(out=xg, in_=x[128 * s:128 * (s + 1), :])
                sbm = wk.tile([128, K_IN], F32, name="sbm", tag="sbm")
                prod = wk.tile([128, TWO_N * D], F32, name="prod", tag="prod")
                for dl in range(TWO_N):
                    npair = TWO_N - dl
                    nc.vector.tensor_mul(
                        prod[:, 0:npair * D],
                        xg[:, 0:npair * D],
                        xg[:, dl * D:(dl + npair) * D],
                    )
                    dst = bass.AP(tensor=sbm.tensor, offset=sbm.offset + dl,
                                  ap=[sbm.ap[0], [17, npair]])
                    nc.vector.tensor_reduce(
                        dst, prod[:, 0:npair * D].rearrange(
                            "p (n d) -> p n d", d=D),
                        axis=mybir.AxisListType.X, op=mybir.AluOpType.add)
                for dl in range(1, TWO_N):
                    npair = TWO_N - dl
                    src = bass.AP(tensor=sbm.tensor, offset=sbm.offset + dl,
                                  ap=[sbm.ap[0], [17, npair]])
                    dst = bass.AP(tensor=sbm.tensor,
                                  offset=sbm.offset + 16 * dl,
                                  ap=[sbm.ap[0], [17, npair]])
                    nc.vector.tensor_copy(out=dst, in_=src)
                for i in range(2):
                    pt = ps.tile([128, 128], F32, name="tp", tag="tp", bufs=2)
                    nc.tensor.transpose(
                        pt[:, :], sbm[:, 128 * i:128 * (i + 1)], ident)
                    nc.vector.tensor_copy(
                        out=scalT[i][:, 128 * s:128 * (s + 1)], in_=pt[:, :])

            # ---------------- Phase B: grouped MLP ----------------
            for l in range(TWO_N):
                w0t = w0p.tile([128, 2, H], F32R, name="w0t", tag="w0")
                nc.sync.dma_start(
                    out=w0t,
                    in_=W0[l, :, :].rearrange("(t p) m -> p t m", p=128))
                w1h = []
                w2h = []
                for hname, Wsrc, lst in (("w1", W1, w1h), ("w2", W2, w2h)):
                    for half in range(2):
                        wt = w12p.tile([128, 4, H], F32R,
                                       name=f"{hname}{half}", tag="w12")
                        nc.sync.dma_start(
                            out=wt,
                            in_=Wsrc[l, 512 * half:512 * (half + 1), :]
                            .rearrange("(t p) m -> p t m", p=128))
                        lst.append(wt)
                w3t = w3p.tile([128, 8, TWO_N], F32R, name="w3t", tag="w3")
                nc.sync.dma_start(
                    out=w3t,
                    in_=W3[l, :, :].rearrange("(t p) m -> p t m", p=128))
                bt012 = bp.tile([128, 3, 8], F32, name="bt012", tag="b012")
                for li, bsrc in enumerate((b0, b1, b2)):
                    nc.sync.dma_start(
                        out=bt012[:, li, :],
                        in_=bsrc[l, :].rearrange("(t p) -> p t", p=128))
                bt3 = bp.tile([16, 1], F32, name="bt3", tag="b3")
                nc.sync.dma_start(out=bt3, in_=b3[l, :].unsqueeze(1))

                for bt in range(N_BT):
                    bs = 512 * bt
                    # L0: scalT -> h0
                    h0 = hp.tile([128, 8, 512], F32R, name="h0", tag="h")
                    for ot in range(8):
                        pt = ps.tile([128, 512], F32, name="mlp", tag="mlp", bufs=4)
                        for kt in range(2):
                            nc.tensor.matmul(
                                pt[:, :],
                                w0t[:, kt, 128 * ot:128 * (ot + 1)],
                                scalT[kt][:, bs:bs + 512],
                                start=(kt == 0), stop=(kt == 1))
                        nc.scalar.activation(
                            h0[:, ot, :], pt[:, :],
                            mybir.ActivationFunctionType.Tanh,
                            bias=bt012[:, 0, ot:ot + 1])
                    # L1, L2
                    hin = h0
                    for li, whalves in ((1, w1h), (2, w2h)):
                        hout = hp.tile([128, 8, 512], F32R,
                                       name=f"h{li}", tag="h")
                        for ot in range(8):
                            pt = ps.tile([128, 512], F32, name="mlp", tag="mlp", bufs=4)
                            for kt in range(8):
                                nc.tensor.matmul(
                                    pt[:, :],
                                    whalves[kt // 4][:, kt % 4,
                                                     128 * ot:128 * (ot + 1)],
                                    hin[:, kt, :],
                                    start=(kt == 0), stop=(kt == 7))
                            nc.scalar.activation(
                                hout[:, ot, :], pt[:, :],
                                mybir.ActivationFunctionType.Tanh,
                                bias=bt012[:, li, ot:ot + 1])
                        hin = hout
                    # L3 -> coeffs [16, 512] + bias, transpose to batch-major
                    p3 = ps.tile([16, 512], F32, name="p3", tag="p3", bufs=2)
                    for kt in range(8):
                        nc.tensor.matmul(p3[:, :], w3t[:, kt, :],
                                         hin[:, kt, :],
                                         start=(kt == 0), stop=(kt == 7))
                    csb = wk.tile([16, 512], F32, name="csb", tag="csb")
                    nc.vector.tensor_scalar_add(csb[:, :], p3[:, :],
                                                bt3[:, 0:1])
                    for j in range(4):
                        tp = ps.tile([128, 16], F32, name="tp2", tag="tp", bufs=2)
                        nc.tensor.transpose(
                            tp[:, 0:16], csb[:, 128 * j:128 * (j + 1)],
                            ident[0:16, 0:16])
                        sub = 4 * bt + j
                        nc.vector.tensor_copy(
                            out=coeff[sub][:, 16 * l:16 * (l + 1)],
                            in_=tp[:, 0:16])

                # final contraction for this l (overlaps next l's GEMMs):
                # y[bsub, l*64+d] = sum_n coeff[b, 16l+n] * x[b, 64n+d]
                for s in range(N_SUB):
                    xg = xgp.tile([128, TWO_N * D], F32, name="xg2", tag="xg")
                    nc.sync.dma_start(out=xg,
                                      in_=x[128 * s:128 * (s + 1), :])
                    prod = wk.tile([128, TWO_N * D], F32, name="prod2",
                                   tag="prod")
                    in0 = bass.AP(tensor=xg.tensor, offset=xg.offset,
                                  ap=[xg.ap[0], [1, D], [D, TWO_N]])
                    c = coeff[s]
                    in1 = bass.AP(tensor=c.tensor, offset=c.offset + 16 * l,
                                  ap=[c.ap[0], [0, D], [1, TWO_N]])
                    out_ap = bass.AP(tensor=prod.tensor, offset=prod.offset,
                                     ap=[prod.ap[0], [16, D], [1, TWO_N]])
                    nc.vector.tensor_mul(out_ap, in0, in1)
                    fcol = finp.tile([128, D], F32, name="fcol", tag="fcol")
                    nc.vector.tensor_reduce(
                        fcol[:, :],
                        prod[:, :].rearrange("p (d n) -> p d n", n=TWO_N),
                        axis=mybir.AxisListType.X, op=mybir.AluOpType.add)
                    nc.sync.dma_start(
                        out=y[128 * s:128 * (s + 1),
                              D * l:D * (l + 1)],
                        in_=fcol[:, :])

    nc.finalize()
    return nc


_NC = None


def kernel(x, W0, b0, W1, b1, W2, b2, W3, b3):
    global _NC
    if _NC is None:
        _NC = _build_program()

    x = np.ascontiguousarray(np.asarray(x, dtype=np.float32))
    shared = {
        "W0": np.ascontiguousarray(np.asarray(W0, np.float32)),
        "W1": np.ascontiguousarray(np.asarray(W1, np.float32)),
        "W2": np.ascontiguousarray(np.asarray(W2, np.float32)),
        "W3": np.ascontiguousarray(np.asarray(W3, np.float32)),
        "b0": np.ascontiguousarray(np.asarray(b0, np.float32)),
        "b1": np.ascontiguousarray(np.asarray(b1, np.float32)),
        "b2": np.ascontiguousarray(np.asarray(b2, np.float32)),
        "b3": np.ascontiguousarray(np.asarray(b3, np.float32)),
    }
    in_maps = []
    for c in range(N_CORES):
        m = dict(shared)
        m["x"] = x[B_LOC * c:B_LOC * (c + 1), :]
        in_maps.append(m)
    res = run_bass_kernel_spmd(_NC, in_maps, list(range(N_CORES)))
    return np.concatenate([res.results[c]["y"] for c in range(N_CORES)],
                          axis=0)
